# revision 43
# baseline (speedup 1.0000x reference)
"""Bass/Trainium2 kernel for 2-layer GAT (nn_GAT_50577534878113).

Strategy (8 NeuronCores, SPMD):
  - Nodes padded to NP = NBLK*128; dst-sorted edges sharded by dst-block range:
    core k owns BPC = NBLK/8 blocks of 128 destination nodes.
  - Dense phase (x@W1) replicated per core in bf16; per-node payload rows
    (h, 256 B each) land in a per-core DRAM table PL1X.
  - Edge phase per dst block: bulk `dma_gather` (stock Q7 gather, int16
    indices, 256 B rows) fetches all edge payloads in two calls (node table
    split at 32768 for the int16 index range).  Per-edge a_src is recomputed
    on DVE from the gathered h (mult + axis-reduce); per-edge a_dst comes from
    a per-chunk TensorE matmul against the transposed one-hot PT.  The
    scatter-add + softmax denominator run as TensorE matmuls with the one-hot
    P as the stationary operand, accumulating [dst, feat|den] in PSUM.
  - Layer-2 local dense fused into the block epilogue; one AllGather
    distributes the compact layer-2 table, which is repacked to 256 B rows;
    the layer-2 edge phase mirrors layer 1 (denominator folded, a_src2
    carried in the row).  Output is node-sharded; host concatenates.

To keep per-core programs identical (SPMD), each core's node table is block-
rotated so its own 49 dst blocks come first; L1 gather indices are rotated to
match. The AllGather (in core order) restores the global node order for L2.
"""

import numpy as np
import ml_dtypes

bf16 = ml_dtypes.bfloat16

# Problem shapes (hardcoded per contract)
N_NODES = 50000
N_EDGES = 800000
IN_CH = 128
HEADS = 4
HIDDEN = 32
OUT_CH = 40
NEG = 0.2
NCORES = 8
BLK = 128
HALF = 32768          # int16 index split point

F1T = IN_CH + 2 * HEADS     # 136: stage-1 dense out [h | a_src1 | a_dst1]
F1E = IN_CH + HEADS         # 132: [msg cols | den cols] in the L1 scatter
F2 = OUT_CH + 2             # 42:  [h2 | a_src2 | a_dst2]
F2E = OUT_CH + 1            # 41:  [msg cols | den col] in the L2 scatter
ROW1 = 256                  # L1 payload row elems (512 B bf16): [h|a_src|pad]
ROW = 128                   # L2 payload row elems (256 B bf16)
F1W = IN_CH + HEADS         # 132: written L1 row prefix [h | a_src]


def _build(NP, NBLK, BPC, CA1, CB1, CA2, CB2):
    import concourse.bass as bass
    import concourse.bacc as bacc
    import concourse.mybir as mybir
    import concourse.tile as tile

    dt = mybir.dt
    AL = mybir.AluOpType
    AF = mybir.ActivationFunctionType
    CS1 = CA1 + CB1
    CS2 = CA2 + CB2

    nc = bacc.Bacc("TRN2", target_bir_lowering=False, debug=False,
                   num_devices=NCORES, num_swdge_queues=4)

    XT = nc.dram_tensor("xt", [128, NP], dt.bfloat16, kind="ExternalInput").ap()
    W1A = nc.dram_tensor("w1a", [128, F1T], dt.bfloat16, kind="ExternalInput").ap()
    W2A = nc.dram_tensor("w2a", [128, F2], dt.bfloat16, kind="ExternalInput").ap()
    IOTA = nc.dram_tensor("iota", [128, 128], dt.uint8, kind="ExternalInput").ap()
    IOTAP = nc.dram_tensor("iotap", [128, 1], dt.uint8, kind="ExternalInput").ap()
    IDB = nc.dram_tensor("idb", [128, 128], dt.bfloat16, kind="ExternalInput").ap()
    IXA1 = nc.dram_tensor("ixa1", [128, BPC * CA1 * 8], dt.int16, kind="ExternalInput").ap()
    IXB1 = nc.dram_tensor("ixb1", [128, BPC * CB1 * 8], dt.int16, kind="ExternalInput").ap()
    IXA2 = nc.dram_tensor("ixa2", [128, BPC * CA2 * 8], dt.int16, kind="ExternalInput").ap()
    IXB2 = nc.dram_tensor("ixb2", [128, BPC * CB2 * 8], dt.int16, kind="ExternalInput").ap()
    DREL1 = nc.dram_tensor("drel1", [128, BPC * CS1], dt.uint8, kind="ExternalInput").ap()
    DREL2 = nc.dram_tensor("drel2", [128, BPC * CS2], dt.uint8, kind="ExternalInput").ap()
    DRT1 = nc.dram_tensor("drt1", [BPC, CS1 * 128], dt.uint8, kind="ExternalInput").ap()
    DRT2 = nc.dram_tensor("drt2", [BPC, CS2 * 128], dt.uint8, kind="ExternalInput").ap()
    OUT = nc.dram_tensor("out", [BPC * 128, OUT_CH], dt.float32, kind="ExternalOutput").ap()

    PL1X = nc.dram_tensor("pl1x", [NP, ROW1], dt.bfloat16).ap()
    L2L = nc.dram_tensor("l2l", [BPC * 128, F2], dt.bfloat16).ap()
    PL2C = nc.dram_tensor("pl2c", [NP, F2], dt.bfloat16, addr_space="Shared").ap()
    PL2X = nc.dram_tensor("pl2x", [NP, ROW], dt.bfloat16).ap()

    with tile.TileContext(nc) as tc:
        with tc.tile_pool(name="const", bufs=1) as cp, \
             tc.tile_pool(name="sb", bufs=3) as sp, \
             tc.tile_pool(name="gat", bufs=5) as gp, \
             tc.tile_pool(name="blk", bufs=2) as bp, \
             tc.tile_pool(name="ps", bufs=2, space="PSUM") as pp:

            iota = cp.tile([128, 128], dt.uint8)
            nc.sync.dma_start(out=iota[:], in_=IOTA[:])
            iotap = cp.tile([128, 1], dt.uint8)
            nc.sync.dma_start(out=iotap[:], in_=IOTAP[:])
            idb = cp.tile([128, 128], dt.bfloat16)
            nc.sync.dma_start(out=idb[:], in_=IDB[:])
            w1a = cp.tile([128, F1T], dt.bfloat16)
            nc.sync.dma_start(out=w1a[:], in_=W1A[:])
            w2a = cp.tile([128, F2], dt.bfloat16)
            nc.sync.dma_start(out=w2a[:], in_=W2A[:])

            ixa1 = cp.tile([128, BPC * CA1 * 8], dt.int16)
            nc.sync.dma_start(out=ixa1[:], in_=IXA1[:])
            ixb1 = cp.tile([128, BPC * CB1 * 8], dt.int16)
            nc.sync.dma_start(out=ixb1[:], in_=IXB1[:])
            ixa2 = cp.tile([128, BPC * CA2 * 8], dt.int16)
            nc.sync.dma_start(out=ixa2[:], in_=IXA2[:])
            ixb2 = cp.tile([128, BPC * CB2 * 8], dt.int16)
            nc.sync.dma_start(out=ixb2[:], in_=IXB2[:])
            drel1 = cp.tile([128, BPC * CS1], dt.uint8)
            nc.sync.dma_start(out=drel1[:], in_=DREL1[:])
            drel2 = cp.tile([128, BPC * CS2], dt.uint8)
            nc.sync.dma_start(out=drel2[:], in_=DREL2[:])

            adst1 = cp.tile([128, BPC * HEADS], dt.bfloat16)
            adst2 = cp.tile([128, BPC], dt.bfloat16)
            elu1t = cp.tile([128, BPC * 128], dt.bfloat16)

            # ---- stage 1: dense h1/a_src1/a_dst1 for ALL nodes; write PL1X ----
            XCH = 8
            for g0 in range(0, NBLK, XCH):
                xt_big = sp.tile([128, XCH * 128], dt.bfloat16, tag="xt")
                nc.sync.dma_start(out=xt_big[:],
                                  in_=XT[:, g0 * 128:(g0 + XCH) * 128])
                wr = sp.tile([128, XCH * F1W], dt.bfloat16, tag="wr")
                for j in range(XCH):
                    nb = g0 + j
                    dps = pp.tile([128, F1T], dt.float32,
                                  tag="agg" if j % 2 == 0 else "adg")
                    nc.tensor.matmul(dps[:], lhsT=xt_big[:, j * 128:(j + 1) * 128],
                                     rhs=w1a[:], start=True, stop=True)
                    eng = nc.vector.tensor_copy if j % 2 else nc.scalar.copy
                    eng(wr[:, j * F1W:(j + 1) * F1W], dps[:, 0:F1W])
                    if nb < BPC:
                        nc.vector.tensor_copy(adst1[:, nb * HEADS:(nb + 1) * HEADS],
                                              dps[:, F1W:F1T])
                dst_view = PL1X[g0 * 128:(g0 + XCH) * 128, 0:F1W] \
                    .rearrange("(x p) f -> p x f", p=128)
                nc.scalar.dma_start(out=dst_view, in_=wr[:].rearrange(
                    "p (x f) -> p x f", f=F1W))

            # ---- stage 2: layer-1 edge pass over own dst blocks ----
            for b in range(BPC):
                g = gp.tile([128, CS1 * ROW1], dt.bfloat16, tag="g")
                gv3 = g[:].rearrange("p (c f) -> p c f", f=ROW1)
                nc.gpsimd.dma_gather(
                    gv3[:, 0:CA1, :], PL1X[0:HALF, :],
                    ixa1[:, b * CA1 * 8:(b + 1) * CA1 * 8],
                    CA1 * 128, CA1 * 128, ROW1, single_packet=False,
                    queue_num=(2 * b) % 4)
                nc.gpsimd.dma_gather(
                    gv3[:, CA1:CS1, :], PL1X[HALF:NP, :],
                    ixb1[:, b * CB1 * 8:(b + 1) * CB1 * 8],
                    CB1 * 128, CB1 * 128, ROW1, single_packet=False,
                    queue_num=(2 * b + 1) % 4)

                # one-hot P[e, c, j] = (dst_rel[e,c] == j) and its transpose
                P = bp.tile([128, CS1 * 128], dt.bfloat16, tag="P")
                nc.vector.tensor_tensor(
                    out=P[:].rearrange("p (c j) -> p c j", j=128),
                    in0=drel1[:, b * CS1:(b + 1) * CS1][:, :, None]
                        .broadcast_to([128, CS1, 128]),
                    in1=iota[:, None, :].broadcast_to([128, CS1, 128]),
                    op=AL.is_equal)
                drt = sp.tile([128, CS1 * 128], dt.uint8, tag="drt")
                nc.sync.dma_start(
                    out=drt[:],
                    in_=DRT1[b:b + 1, :].broadcast_to([128, CS1 * 128]))
                PT = bp.tile([128, CS1 * 128], dt.bfloat16, tag="PT")
                nc.vector.tensor_tensor(
                    out=PT[:].rearrange("p (c e) -> p c e", e=128),
                    in0=drt[:].rearrange("p (c e) -> p c e", e=128),
                    in1=iotap[:, :, None].broadcast_to([128, CS1, 128]),
                    op=AL.is_equal)

                # per-edge a_dst via PT-matmul; a_src carried in the row
                ADG = pp.tile([128, CS1 * HEADS], dt.float32, tag="adg")
                for c in range(CS1):
                    nc.tensor.matmul(ADG[:, c * HEADS:(c + 1) * HEADS],
                                     lhsT=PT[:, c * 128:(c + 1) * 128],
                                     rhs=adst1[:, b * HEADS:(b + 1) * HEADS],
                                     start=True, stop=True)

                # logits -> leaky relu -> exp
                elog = sp.tile([128, CS1 * HEADS], dt.float32, tag="elog")
                nc.vector.tensor_tensor(
                    out=elog[:].rearrange("p (c h) -> p c h", h=HEADS),
                    in0=gv3[:, :, IN_CH:IN_CH + HEADS],
                    in1=ADG[:].rearrange("p (c h) -> p c h", h=HEADS),
                    op=AL.add)
                lr = sp.tile([128, CS1 * HEADS], dt.float32, tag="lr")
                nc.vector.scalar_tensor_tensor(out=lr[:], in0=elog[:], scalar=NEG,
                                               in1=elog[:], op0=AL.mult, op1=AL.max)
                s_all = sp.tile([128, CS1 * HEADS], dt.bfloat16, tag="sall")
                nc.scalar.activation(out=s_all[:], in_=lr[:], func=AF.Exp)
                sv = s_all[:].rearrange("p (c h) -> p c h", h=HEADS)

                # gs[e, c, 0:128] = h_src * exp-logit
                gs = bp.tile([128, CS1 * IN_CH], dt.bfloat16, tag="gs")
                nc.vector.tensor_tensor(
                    out=gs[:].rearrange("p (c h w) -> p c h w",
                                        h=HEADS, w=HIDDEN),
                    in0=g[:].rearrange("p (c h w) -> p c h w",
                                       h=ROW1 // HIDDEN, w=HIDDEN)[:, :, 0:HEADS, :],
                    in1=sv[:, :, :, None].broadcast_to(
                        [128, CS1, HEADS, HIDDEN]),
                    op=AL.mult)

                # AGGT[j, 0:128] = msg sums; DENT[j, 0:4] = denominators
                AGGT = pp.tile([128, IN_CH], dt.float32, tag="agg")
                DENT = pp.tile([128, HEADS], dt.float32, tag="adg")
                for c in range(CS1):
                    nc.tensor.matmul(AGGT[:],
                                     lhsT=P[:, c * 128:(c + 1) * 128],
                                     rhs=gs[:, c * IN_CH:(c + 1) * IN_CH],
                                     start=(c == 0), stop=(c == CS1 - 1))
                    nc.tensor.matmul(DENT[:],
                                     lhsT=P[:, c * 128:(c + 1) * 128],
                                     rhs=s_all[:, c * HEADS:(c + 1) * HEADS],
                                     start=(c == 0), stop=(c == CS1 - 1))

                # normalize (per-head per-partition scalar) + ELU in [j, f]
                rec = sp.tile([128, HEADS], dt.float32, tag="rec")
                nc.vector.tensor_scalar(out=rec[:], in0=DENT[:],
                                        scalar1=1e-16, scalar2=None, op0=AL.add)
                nc.vector.reciprocal_approx_fast(out=rec[:], in_=rec[:])
                t1 = sp.tile([128, 128], dt.float32, tag="t1")
                for h in range(HEADS):
                    nc.vector.tensor_scalar(
                        out=t1[:, h * HIDDEN:(h + 1) * HIDDEN],
                        in0=AGGT[:, h * HIDDEN:(h + 1) * HIDDEN],
                        scalar1=rec[:, h:h + 1], scalar2=None, op0=AL.mult)
                rn = sp.tile([128, 128], dt.float32, tag="rn")
                nc.scalar.activation(out=rn[:], in_=t1[:], func=AF.Relu, scale=-1.0)
                u1 = sp.tile([128, 128], dt.float32, tag="u1")
                nc.scalar.activation(out=u1[:], in_=rn[:], func=AF.Exp, scale=-1.0)
                ejf = sp.tile([128, 128], dt.bfloat16, tag="ejf")
                nc.vector.scalar_tensor_tensor(out=ejf[:], in0=u1[:], scalar=-1.0,
                                               in1=t1[:], op0=AL.add, op1=AL.max)

                # transpose [j, f] -> [f, j] for the layer-2 dense lhsT
                etp = pp.tile([128, 128], dt.bfloat16, tag="etp")
                nc.tensor.transpose(out=etp[:], in_=ejf[:], identity=idb[:])
                nc.scalar.copy(out=elu1t[:, b * 128:(b + 1) * 128], in_=etp[:])

                # layer-2 local dense for this block (fused stage 3)
                d2 = pp.tile([128, F2], dt.float32, tag="d2")
                nc.tensor.matmul(d2[:], lhsT=elu1t[:, b * 128:(b + 1) * 128],
                                 rhs=w2a[:], start=True, stop=True)
                sb2 = sp.tile([128, F2], dt.bfloat16, tag="sb2")
                nc.scalar.copy(out=sb2[:], in_=d2[:])
                nc.scalar.dma_start(out=L2L[b * 128:(b + 1) * 128, :], in_=sb2[:])
                nc.vector.tensor_copy(adst2[:, b:b + 1], d2[:, F2 - 1:F2])

            nc.gpsimd.collective_compute(
                "AllGather", mybir.AluOpType.bypass,
                replica_groups=[list(range(NCORES))],
                ins=[L2L[:, :]], outs=[PL2C[:, :]])

            # repack compact [NP, 42] -> 256 B rows [NP, 128] via SBUF bounce
            for gg in range(NCORES):
                r0 = gg * BPC * 128
                pk = sp.tile([128, BPC * F2], dt.bfloat16, tag="pk")
                nc.sync.dma_start(
                    out=pk[:].rearrange("p (x f) -> p x f", f=F2),
                    in_=PL2C[r0:r0 + BPC * 128, :]
                        .rearrange("(x p) f -> p x f", p=128))
                nc.scalar.dma_start(
                    out=PL2X[r0:r0 + BPC * 128, 0:F2]
                        .rearrange("(x p) f -> p x f", p=128),
                    in_=pk[:].rearrange("p (x f) -> p x f", f=F2))

            # ---- stage 4: layer-2 edge pass ----
            for b in range(BPC):
                g2 = gp.tile([128, CS2 * ROW], dt.bfloat16, tag="g2")
                g2v = g2[:].rearrange("p (c f) -> p c f", f=ROW)
                nc.gpsimd.dma_gather(
                    g2v[:, 0:CA2, :], PL2X[0:HALF, :],
                    ixa2[:, b * CA2 * 8:(b + 1) * CA2 * 8],
                    CA2 * 128, CA2 * 128, ROW, single_packet=False,
                    queue_num=(2 * b) % 4)
                nc.gpsimd.dma_gather(
                    g2v[:, CA2:CS2, :], PL2X[HALF:NP, :],
                    ixb2[:, b * CB2 * 8:(b + 1) * CB2 * 8],
                    CB2 * 128, CB2 * 128, ROW, single_packet=False,
                    queue_num=(2 * b + 1) % 4)

                P = bp.tile([128, CS2 * 128], dt.bfloat16, tag="P")
                nc.vector.tensor_tensor(
                    out=P[:].rearrange("p (c j) -> p c j", j=128),
                    in0=drel2[:, b * CS2:(b + 1) * CS2][:, :, None]
                        .broadcast_to([128, CS2, 128]),
                    in1=iota[:, None, :].broadcast_to([128, CS2, 128]),
                    op=AL.is_equal)
                drt = sp.tile([128, CS2 * 128], dt.uint8, tag="drt")
                nc.sync.dma_start(
                    out=drt[:],
                    in_=DRT2[b:b + 1, :].broadcast_to([128, CS2 * 128]))
                PT = bp.tile([128, CS2 * 128], dt.bfloat16, tag="PT")
                nc.vector.tensor_tensor(
                    out=PT[:].rearrange("p (c e) -> p c e", e=128),
                    in0=drt[:].rearrange("p (c e) -> p c e", e=128),
                    in1=iotap[:, :, None].broadcast_to([128, CS2, 128]),
                    op=AL.is_equal)

                ADG2 = pp.tile([128, CS2], dt.float32, tag="adg")
                for c in range(CS2):
                    nc.tensor.matmul(ADG2[:, c:c + 1],
                                     lhsT=PT[:, c * 128:(c + 1) * 128],
                                     rhs=adst2[:, b:b + 1],
                                     start=True, stop=True)

                elog2 = sp.tile([128, CS2], dt.float32, tag="elog2")
                nc.vector.tensor_tensor(
                    out=elog2[:, :, None], in0=ADG2[:, :, None],
                    in1=g2v[:, :, OUT_CH:OUT_CH + 1], op=AL.add)
                lr2 = sp.tile([128, CS2], dt.float32, tag="lr2")
                nc.vector.scalar_tensor_tensor(out=lr2[:], in0=elog2[:], scalar=NEG,
                                               in1=elog2[:], op0=AL.mult, op1=AL.max)
                s2 = sp.tile([128, CS2], dt.bfloat16, tag="s2")
                nc.scalar.activation(out=s2[:], in_=lr2[:], func=AF.Exp)

                gs2 = bp.tile([128, CS2 * OUT_CH], dt.bfloat16, tag="gs2")
                nc.vector.tensor_tensor(
                    out=gs2[:].rearrange("p (c f) -> p c f", f=OUT_CH),
                    in0=g2v[:, :, 0:OUT_CH],
                    in1=s2[:, :, None].broadcast_to([128, CS2, OUT_CH]),
                    op=AL.mult)

                AGG2T = pp.tile([128, OUT_CH], dt.float32, tag="agg")
                DEN2T = pp.tile([128, 1], dt.float32, tag="adg")
                for c in range(CS2):
                    nc.tensor.matmul(AGG2T[:],
                                     lhsT=P[:, c * 128:(c + 1) * 128],
                                     rhs=gs2[:, c * OUT_CH:(c + 1) * OUT_CH],
                                     start=(c == 0), stop=(c == CS2 - 1))
                    nc.tensor.matmul(DEN2T[:],
                                     lhsT=P[:, c * 128:(c + 1) * 128],
                                     rhs=s2[:, c:c + 1],
                                     start=(c == 0), stop=(c == CS2 - 1))

                rec2 = sp.tile([128, 1], dt.float32, tag="rec2")
                nc.vector.tensor_scalar(out=rec2[:], in0=DEN2T[:],
                                        scalar1=1e-16, scalar2=None, op0=AL.add)
                nc.vector.reciprocal_approx_fast(out=rec2[:], in_=rec2[:])
                o2 = sp.tile([128, OUT_CH], dt.float32, tag="o2")
                nc.vector.tensor_scalar(out=o2[:], in0=AGG2T[:],
                                        scalar1=rec2[:, 0:1], scalar2=None,
                                        op0=AL.mult)
                nc.sync.dma_start(out=OUT[b * 128:(b + 1) * 128, :], in_=o2[:])

    nc.compile()
    return nc


def _wrap_idx(lst):
    """list [n] (n % 128 == 0) -> [128, n//16] int16, wrapped + replicated."""
    S = len(lst) // 16
    w = np.asarray(lst, np.int16).reshape(S, 16).T       # [16, S]
    return np.tile(w, (8, 1))                            # [128, S]


def _host_prep(x, edge_index, W1, att_src1, att_dst1, W2, att_src2, att_dst2,
               n_nodes, n_edges):
    NBLK = -(-n_nodes // BLK)
    NBLK = -(-NBLK // NCORES) * NCORES
    NP = NBLK * BLK
    BPC = NBLK // NCORES

    x = np.asarray(x, np.float32)
    W1 = np.asarray(W1, np.float32)
    W2 = np.asarray(W2, np.float32)
    att_src1 = np.asarray(att_src1, np.float32)
    att_dst1 = np.asarray(att_dst1, np.float32)
    att_src2 = np.asarray(att_src2, np.float32)
    att_dst2 = np.asarray(att_dst2, np.float32)
    H, C = att_src1.shape

    xp = np.zeros((NP, IN_CH), np.float32)
    xp[:n_nodes] = x
    XT = np.ascontiguousarray(xp.T).astype(bf16)          # [128, NP]

    Asrc1 = np.zeros((H * C, H), np.float32)
    Adst1 = np.zeros((H * C, H), np.float32)
    for h in range(H):
        Asrc1[h * C:(h + 1) * C, h] = att_src1[h]
        Adst1[h * C:(h + 1) * C, h] = att_dst1[h]
    W1A = np.concatenate([W1, W1 @ Asrc1, W1 @ Adst1], axis=1).astype(bf16)
    W2A = np.concatenate([W2, W2 @ att_src2.T, W2 @ att_dst2.T], axis=1).astype(bf16)

    IOTA = np.ascontiguousarray(
        np.broadcast_to(np.arange(128, dtype=np.uint8), (128, 128)))
    IOTAP = np.arange(128, dtype=np.uint8)[:, None]
    IDB = np.eye(128, dtype=np.float32).astype(bf16)

    src = np.asarray(edge_index[0], np.int64)
    dst = np.asarray(edge_index[1], np.int64)
    order = np.lexsort((src, dst))
    ss = src[order]
    dd = dst[order]
    blk = dd // BLK
    core_of = blk // BPC
    b_of = blk % BPC
    d_rel = dd % BLK

    # per (core, block): L1 split on rotated ids, L2 split on global ids
    def rot_of(k, s):
        return ((s // BLK - k * BPC) % NBLK) * BLK + (s % BLK)

    # first pass: chunk maxima
    kA1 = np.zeros((NCORES, BPC), np.int64)
    kB1 = np.zeros((NCORES, BPC), np.int64)
    kA2 = np.zeros((NCORES, BPC), np.int64)
    kB2 = np.zeros((NCORES, BPC), np.int64)
    sel_cache = {}
    for k in range(NCORES):
        for b in range(BPC):
            m = (core_of == k) & (b_of == b)
            sel_cache[(k, b)] = m
            r = rot_of(k, ss[m])
            nA = int((r < HALF).sum()); nB = int(m.sum()) - nA
            kA1[k, b] = -(-nA // 128) if nA else 0
            kB1[k, b] = -(-nB // 128) if nB else 0
            nA2 = int((ss[m] < HALF).sum()); nB2 = int(m.sum()) - nA2
            kA2[k, b] = -(-nA2 // 128) if nA2 else 0
            kB2[k, b] = -(-nB2 // 128) if nB2 else 0
    CA1, CB1 = int(kA1.max()), int(kB1.max())
    CA2, CB2 = int(kA2.max()), int(kB2.max())
    CS1, CS2 = CA1 + CB1, CA2 + CB2

    IXA1 = np.zeros((NCORES, 128, BPC * CA1 * 8), np.int16)
    IXB1 = np.zeros((NCORES, 128, BPC * CB1 * 8), np.int16)
    IXA2 = np.zeros((NCORES, 128, BPC * CA2 * 8), np.int16)
    IXB2 = np.zeros((NCORES, 128, BPC * CB2 * 8), np.int16)
    DREL1 = np.full((NCORES, 128, BPC * CS1), 200, np.uint8)
    DREL2 = np.full((NCORES, 128, BPC * CS2), 200, np.uint8)
    DRT1 = np.full((NCORES, BPC, CS1 * 128), 200, np.uint8)
    DRT2 = np.full((NCORES, BPC, CS2 * 128), 200, np.uint8)
    XTs = []

    for k in range(NCORES):
        XTb = XT.reshape(128, NBLK, BLK)
        XTs.append(np.ascontiguousarray(
            np.roll(XTb, -k * BPC, axis=1).reshape(128, NP)))
        for b in range(BPC):
            m = sel_cache[(k, b)]
            s_k = ss[m]; dr = d_rel[m]
            rot = rot_of(k, s_k)
            # layer 1 (rotated split)
            a_m = rot < HALF
            for (mask, tbl_off, ix_arr, ca, c0) in (
                    (a_m, 0, IXA1, CA1, 0), (~a_m, HALF, IXB1, CB1, CA1)):
                idxs = rot[mask] - tbl_off
                drs = dr[mask]
                n = len(idxs)
                lst = np.zeros(ca * 128, np.int64)
                lst[:n] = idxs
                if ca:
                    ix_arr[k][:, b * ca * 8:(b + 1) * ca * 8] = _wrap_idx(lst)
                sl = np.arange(n)
                pp_, cc = sl % 128, sl // 128 + c0
                DREL1[k][pp_, b * CS1 + cc] = drs
                DRT1[k][b, cc * 128 + pp_] = drs
            # layer 2 (global split)
            a_m2 = s_k < HALF
            for (mask, tbl_off, ix_arr, ca, c0) in (
                    (a_m2, 0, IXA2, CA2, 0), (~a_m2, HALF, IXB2, CB2, CA2)):
                idxs = s_k[mask] - tbl_off
                drs = dr[mask]
                n = len(idxs)
                lst = np.zeros(ca * 128, np.int64)
                lst[:n] = idxs
                if ca:
                    ix_arr[k][:, b * ca * 8:(b + 1) * ca * 8] = _wrap_idx(lst)
                sl = np.arange(n)
                pp_, cc = sl % 128, sl // 128 + c0
                DREL2[k][pp_, b * CS2 + cc] = drs
                DRT2[k][b, cc * 128 + pp_] = drs

    consts = dict(w1a=W1A, w2a=W2A, iota=IOTA, iotap=IOTAP, idb=IDB)
    in_maps = []
    for k in range(NCORES):
        m = dict(consts)
        m["xt"] = XTs[k]
        m["ixa1"] = IXA1[k]; m["ixb1"] = IXB1[k]
        m["ixa2"] = IXA2[k]; m["ixb2"] = IXB2[k]
        m["drel1"] = DREL1[k]
        m["drel2"] = DREL2[k]
        m["drt1"] = DRT1[k]
        m["drt2"] = DRT2[k]
        in_maps.append(m)
    return NP, NBLK, BPC, (CA1, CB1, CA2, CB2), in_maps


_CACHE = {}


def _run(x, edge_index, W1, att_src1, att_dst1, W2, att_src2, att_dst2,
         n_nodes, n_edges, trace=False):
    from concourse import bass_utils
    NP, NBLK, BPC, CS, in_maps = _host_prep(
        x, edge_index, W1, att_src1, att_dst1, W2, att_src2, att_dst2,
        n_nodes, n_edges)
    key = (NP, CS)
    if key not in _CACHE:
        _CACHE[key] = _build(NP, NBLK, BPC, *CS)
    nc = _CACHE[key]
    res = bass_utils.run_bass_kernel_spmd(nc, in_maps, core_ids=list(range(NCORES)),
                                          trace=trace)
    out = np.concatenate([np.asarray(res.results[k]["out"]) for k in range(NCORES)],
                         axis=0)[:n_nodes]
    return np.ascontiguousarray(out.astype(np.float32)), res


def kernel(x, edge_index, W1, att_src1, att_dst1, W2, att_src2, att_dst2):
    out, _ = _run(x, edge_index, W1, att_src1, att_dst1, W2, att_src2, att_dst2,
                  N_NODES, N_EDGES)
    return out


# revision 47
# speedup vs baseline: 1.0058x; 1.0058x over previous
"""Bass/Trainium2 kernel for 2-layer GAT (nn_GAT_50577534878113).

Strategy (8 NeuronCores, SPMD):
  - Nodes padded to NP = NBLK*128; dst-sorted edges sharded by dst-block range:
    core k owns BPC = NBLK/8 blocks of 128 destination nodes.
  - Dense phase (x@W1) replicated per core in bf16; per-node payload rows
    (h, 256 B each) land in a per-core DRAM table PL1X.
  - Edge phase per dst block: bulk `dma_gather` (stock Q7 gather, int16
    indices, 256 B rows) fetches all edge payloads in two calls (node table
    split at 32768 for the int16 index range).  Per-edge a_src is recomputed
    on DVE from the gathered h (mult + axis-reduce); per-edge a_dst comes from
    a per-chunk TensorE matmul against the transposed one-hot PT.  The
    scatter-add + softmax denominator run as TensorE matmuls with the one-hot
    P as the stationary operand, accumulating [dst, feat|den] in PSUM.
  - Layer-2 local dense fused into the block epilogue; one AllGather
    distributes the compact layer-2 table, which is repacked to 256 B rows;
    the layer-2 edge phase mirrors layer 1 (denominator folded, a_src2
    carried in the row).  Output is node-sharded; host concatenates.

To keep per-core programs identical (SPMD), each core's node table is block-
rotated so its own 49 dst blocks come first; L1 gather indices are rotated to
match. The AllGather (in core order) restores the global node order for L2.
"""

import numpy as np
import ml_dtypes

bf16 = ml_dtypes.bfloat16

# Problem shapes (hardcoded per contract)
N_NODES = 50000
N_EDGES = 800000
IN_CH = 128
HEADS = 4
HIDDEN = 32
OUT_CH = 40
NEG = 0.2
NCORES = 8
BLK = 128
HALF = 32768          # int16 index split point

F1T = IN_CH + 2 * HEADS     # 136: stage-1 dense out [h | a_src1 | a_dst1]
F1E = IN_CH + HEADS         # 132: [msg cols | den cols] in the L1 scatter
F2 = OUT_CH + 2             # 42:  [h2 | a_src2 | a_dst2]
F2E = OUT_CH + 1            # 41:  [msg cols | den col] in the L2 scatter
ROW1 = 256                  # L1 payload row elems (512 B bf16): [h|a_src|pad]
ROW = 128                   # L2 payload row elems (256 B bf16)
F1W = IN_CH + HEADS         # 132: written L1 row prefix [h | a_src]


def _build(NP, NBLK, BPC, CA1, CB1, CA2, CB2):
    import concourse.bass as bass
    import concourse.bacc as bacc
    import concourse.mybir as mybir
    import concourse.tile as tile

    dt = mybir.dt
    AL = mybir.AluOpType
    AF = mybir.ActivationFunctionType
    CS1 = CA1 + CB1
    CS2 = CA2 + CB2

    nc = bacc.Bacc("TRN2", target_bir_lowering=False, debug=False,
                   num_devices=NCORES, num_swdge_queues=4)

    XT = nc.dram_tensor("xt", [128, NP], dt.bfloat16, kind="ExternalInput").ap()
    W1A = nc.dram_tensor("w1a", [128, F1T], dt.bfloat16, kind="ExternalInput").ap()
    W2A = nc.dram_tensor("w2a", [128, F2], dt.bfloat16, kind="ExternalInput").ap()
    IOTA = nc.dram_tensor("iota", [128, 128], dt.uint8, kind="ExternalInput").ap()
    IOTAP = nc.dram_tensor("iotap", [128, 1], dt.uint8, kind="ExternalInput").ap()
    IDB = nc.dram_tensor("idb", [128, 128], dt.bfloat16, kind="ExternalInput").ap()
    IXA1 = nc.dram_tensor("ixa1", [128, BPC * CA1 * 8], dt.int16, kind="ExternalInput").ap()
    IXB1 = nc.dram_tensor("ixb1", [128, BPC * CB1 * 8], dt.int16, kind="ExternalInput").ap()
    IXA2 = nc.dram_tensor("ixa2", [128, BPC * CA2 * 8], dt.int16, kind="ExternalInput").ap()
    IXB2 = nc.dram_tensor("ixb2", [128, BPC * CB2 * 8], dt.int16, kind="ExternalInput").ap()
    DREL1 = nc.dram_tensor("drel1", [128, BPC * CS1], dt.uint8, kind="ExternalInput").ap()
    DREL2 = nc.dram_tensor("drel2", [128, BPC * CS2], dt.uint8, kind="ExternalInput").ap()
    DRT1 = nc.dram_tensor("drt1", [BPC, CS1 * 128], dt.uint8, kind="ExternalInput").ap()
    DRT2 = nc.dram_tensor("drt2", [BPC, CS2 * 128], dt.uint8, kind="ExternalInput").ap()
    OUT = nc.dram_tensor("out", [BPC * 128, OUT_CH], dt.float32, kind="ExternalOutput").ap()

    PL1XA = nc.dram_tensor("pl1xa", [HALF, ROW1], dt.bfloat16).ap()
    PL1XB = nc.dram_tensor("pl1xb", [NP - HALF, ROW1], dt.bfloat16).ap()
    L2L = nc.dram_tensor("l2l", [BPC * 128, F2], dt.bfloat16).ap()
    PL2C = nc.dram_tensor("pl2c", [NP, F2], dt.bfloat16, addr_space="Shared").ap()
    PL2X = nc.dram_tensor("pl2x", [NP, ROW], dt.bfloat16).ap()

    with tile.TileContext(nc) as tc:
        with tc.tile_pool(name="const", bufs=1) as cp, \
             tc.tile_pool(name="sb", bufs=3) as sp, \
             tc.tile_pool(name="gat", bufs=5) as gp, \
             tc.tile_pool(name="blk", bufs=2) as bp, \
             tc.tile_pool(name="ps", bufs=2, space="PSUM") as pp:

            iota = cp.tile([128, 128], dt.uint8)
            nc.sync.dma_start(out=iota[:], in_=IOTA[:])
            iotap = cp.tile([128, 1], dt.uint8)
            nc.sync.dma_start(out=iotap[:], in_=IOTAP[:])
            idb = cp.tile([128, 128], dt.bfloat16)
            nc.sync.dma_start(out=idb[:], in_=IDB[:])
            w1a = cp.tile([128, F1T], dt.bfloat16)
            nc.sync.dma_start(out=w1a[:], in_=W1A[:])
            w2a = cp.tile([128, F2], dt.bfloat16)
            nc.sync.dma_start(out=w2a[:], in_=W2A[:])

            ixa1 = cp.tile([128, BPC * CA1 * 8], dt.int16)
            nc.sync.dma_start(out=ixa1[:], in_=IXA1[:])
            ixb1 = cp.tile([128, BPC * CB1 * 8], dt.int16)
            nc.sync.dma_start(out=ixb1[:], in_=IXB1[:])
            ixa2 = cp.tile([128, BPC * CA2 * 8], dt.int16)
            nc.sync.dma_start(out=ixa2[:], in_=IXA2[:])
            ixb2 = cp.tile([128, BPC * CB2 * 8], dt.int16)
            nc.sync.dma_start(out=ixb2[:], in_=IXB2[:])
            drel1 = cp.tile([128, BPC * CS1], dt.uint8)
            nc.sync.dma_start(out=drel1[:], in_=DREL1[:])
            drel2 = cp.tile([128, BPC * CS2], dt.uint8)
            nc.sync.dma_start(out=drel2[:], in_=DREL2[:])

            adst1 = cp.tile([128, BPC * HEADS], dt.bfloat16)
            adst2 = cp.tile([128, BPC], dt.bfloat16)
            elu1t = cp.tile([128, BPC * 128], dt.bfloat16)

            # ---- stage 1: dense h1/a_src1/a_dst1 for ALL nodes; write PL1X ----
            XCH = 8
            for g0 in range(0, NBLK, XCH):
                xt_big = sp.tile([128, XCH * 128], dt.bfloat16, tag="xt")
                nc.sync.dma_start(out=xt_big[:],
                                  in_=XT[:, g0 * 128:(g0 + XCH) * 128])
                wr = sp.tile([128, XCH * F1W], dt.bfloat16, tag="wr")
                for j in range(XCH):
                    nb = g0 + j
                    dps = pp.tile([128, F1T], dt.float32,
                                  tag="agg" if j % 2 == 0 else "adg")
                    nc.tensor.matmul(dps[:], lhsT=xt_big[:, j * 128:(j + 1) * 128],
                                     rhs=w1a[:], start=True, stop=True)
                    eng = nc.vector.tensor_copy if j % 2 else nc.scalar.copy
                    eng(wr[:, j * F1W:(j + 1) * F1W], dps[:, 0:F1W])
                    if nb < BPC:
                        nc.vector.tensor_copy(adst1[:, nb * HEADS:(nb + 1) * HEADS],
                                              dps[:, F1W:F1T])
                r0 = g0 * 128
                tbl = PL1XA if r0 < HALF else PL1XB
                ro = r0 if r0 < HALF else r0 - HALF
                dst_view = tbl[ro:ro + XCH * 128, 0:F1W] \
                    .rearrange("(x p) f -> p x f", p=128)
                nc.scalar.dma_start(out=dst_view, in_=wr[:].rearrange(
                    "p (x f) -> p x f", f=F1W))

            # ---- stage 2: layer-1 edge pass over own dst blocks ----
            for b in range(BPC):
                g = gp.tile([128, CS1 * ROW1], dt.bfloat16, tag="g")
                gv3 = g[:].rearrange("p (c f) -> p c f", f=ROW1)
                nc.gpsimd.dma_gather(
                    gv3[:, 0:CA1, :], PL1XA[:, :],
                    ixa1[:, b * CA1 * 8:(b + 1) * CA1 * 8],
                    CA1 * 128, CA1 * 128, ROW1, single_packet=False,
                    queue_num=(2 * b) % 4)
                nc.gpsimd.dma_gather(
                    gv3[:, CA1:CS1, :], PL1XB[:, :],
                    ixb1[:, b * CB1 * 8:(b + 1) * CB1 * 8],
                    CB1 * 128, CB1 * 128, ROW1, single_packet=False,
                    queue_num=(2 * b + 1) % 4)

                # one-hot P[e, c, j] = (dst_rel[e,c] == j) and its transpose
                P = bp.tile([128, CS1 * 128], dt.bfloat16, tag="P")
                nc.vector.tensor_tensor(
                    out=P[:].rearrange("p (c j) -> p c j", j=128),
                    in0=drel1[:, b * CS1:(b + 1) * CS1][:, :, None]
                        .broadcast_to([128, CS1, 128]),
                    in1=iota[:, None, :].broadcast_to([128, CS1, 128]),
                    op=AL.is_equal)
                drt = sp.tile([128, CS1 * 128], dt.uint8, tag="drt")
                nc.sync.dma_start(
                    out=drt[:],
                    in_=DRT1[b:b + 1, :].broadcast_to([128, CS1 * 128]))
                PT = bp.tile([128, CS1 * 128], dt.bfloat16, tag="PT")
                nc.vector.tensor_tensor(
                    out=PT[:].rearrange("p (c e) -> p c e", e=128),
                    in0=drt[:].rearrange("p (c e) -> p c e", e=128),
                    in1=iotap[:, :, None].broadcast_to([128, CS1, 128]),
                    op=AL.is_equal)

                # per-edge a_dst via PT-matmul; a_src carried in the row
                ADG = pp.tile([128, CS1 * HEADS], dt.float32, tag="adg")
                for c in range(CS1):
                    nc.tensor.matmul(ADG[:, c * HEADS:(c + 1) * HEADS],
                                     lhsT=PT[:, c * 128:(c + 1) * 128],
                                     rhs=adst1[:, b * HEADS:(b + 1) * HEADS],
                                     start=True, stop=True)

                # logits -> leaky relu -> exp
                elog = sp.tile([128, CS1 * HEADS], dt.float32, tag="elog")
                nc.vector.tensor_tensor(
                    out=elog[:].rearrange("p (c h) -> p c h", h=HEADS),
                    in0=gv3[:, :, IN_CH:IN_CH + HEADS],
                    in1=ADG[:].rearrange("p (c h) -> p c h", h=HEADS),
                    op=AL.add)
                lr = sp.tile([128, CS1 * HEADS], dt.float32, tag="lr")
                nc.vector.scalar_tensor_tensor(out=lr[:], in0=elog[:], scalar=NEG,
                                               in1=elog[:], op0=AL.mult, op1=AL.max)
                s_all = sp.tile([128, CS1 * HEADS], dt.bfloat16, tag="sall")
                nc.scalar.activation(out=s_all[:], in_=lr[:], func=AF.Exp)
                sv = s_all[:].rearrange("p (c h) -> p c h", h=HEADS)

                # gs[e, c, 0:128] = h_src * exp-logit
                gs = bp.tile([128, CS1 * IN_CH], dt.bfloat16, tag="gs")
                nc.vector.tensor_tensor(
                    out=gs[:].rearrange("p (c h w) -> p c h w",
                                        h=HEADS, w=HIDDEN),
                    in0=g[:].rearrange("p (c h w) -> p c h w",
                                       h=ROW1 // HIDDEN, w=HIDDEN)[:, :, 0:HEADS, :],
                    in1=sv[:, :, :, None].broadcast_to(
                        [128, CS1, HEADS, HIDDEN]),
                    op=AL.mult)

                # AGGT[j, 0:128] = msg sums; DENT[j, 0:4] = denominators
                AGGT = pp.tile([128, IN_CH], dt.float32, tag="agg")
                DENT = pp.tile([128, HEADS], dt.float32, tag="adg")
                for c in range(CS1):
                    nc.tensor.matmul(AGGT[:],
                                     lhsT=P[:, c * 128:(c + 1) * 128],
                                     rhs=gs[:, c * IN_CH:(c + 1) * IN_CH],
                                     start=(c == 0), stop=(c == CS1 - 1))
                    nc.tensor.matmul(DENT[:],
                                     lhsT=P[:, c * 128:(c + 1) * 128],
                                     rhs=s_all[:, c * HEADS:(c + 1) * HEADS],
                                     start=(c == 0), stop=(c == CS1 - 1))

                # normalize (per-head per-partition scalar) + ELU in [j, f]
                rec = sp.tile([128, HEADS], dt.float32, tag="rec")
                nc.vector.tensor_scalar(out=rec[:], in0=DENT[:],
                                        scalar1=1e-16, scalar2=None, op0=AL.add)
                nc.vector.reciprocal_approx_fast(out=rec[:], in_=rec[:])
                t1 = sp.tile([128, 128], dt.float32, tag="t1")
                for h in range(HEADS):
                    nc.vector.tensor_scalar(
                        out=t1[:, h * HIDDEN:(h + 1) * HIDDEN],
                        in0=AGGT[:, h * HIDDEN:(h + 1) * HIDDEN],
                        scalar1=rec[:, h:h + 1], scalar2=None, op0=AL.mult)
                rn = sp.tile([128, 128], dt.float32, tag="rn")
                nc.scalar.activation(out=rn[:], in_=t1[:], func=AF.Relu, scale=-1.0)
                u1 = sp.tile([128, 128], dt.float32, tag="u1")
                nc.scalar.activation(out=u1[:], in_=rn[:], func=AF.Exp, scale=-1.0)
                ejf = sp.tile([128, 128], dt.bfloat16, tag="ejf")
                nc.vector.scalar_tensor_tensor(out=ejf[:], in0=u1[:], scalar=-1.0,
                                               in1=t1[:], op0=AL.add, op1=AL.max)

                # transpose [j, f] -> [f, j] for the layer-2 dense lhsT
                etp = pp.tile([128, 128], dt.bfloat16, tag="etp")
                nc.tensor.transpose(out=etp[:], in_=ejf[:], identity=idb[:])
                nc.scalar.copy(out=elu1t[:, b * 128:(b + 1) * 128], in_=etp[:])

                # layer-2 local dense for this block (fused stage 3)
                d2 = pp.tile([128, F2], dt.float32, tag="d2")
                nc.tensor.matmul(d2[:], lhsT=elu1t[:, b * 128:(b + 1) * 128],
                                 rhs=w2a[:], start=True, stop=True)
                sb2 = sp.tile([128, F2], dt.bfloat16, tag="sb2")
                nc.scalar.copy(out=sb2[:], in_=d2[:])
                nc.scalar.dma_start(out=L2L[b * 128:(b + 1) * 128, :], in_=sb2[:])
                nc.vector.tensor_copy(adst2[:, b:b + 1], d2[:, F2 - 1:F2])

            nc.gpsimd.collective_compute(
                "AllGather", mybir.AluOpType.bypass,
                replica_groups=[list(range(NCORES))],
                ins=[L2L[:, :]], outs=[PL2C[:, :]])

            # repack compact [NP, 42] -> 256 B rows [NP, 128] via SBUF bounce
            for gg in range(NCORES):
                r0 = gg * BPC * 128
                pk = sp.tile([128, BPC * F2], dt.bfloat16, tag="pk")
                nc.sync.dma_start(
                    out=pk[:].rearrange("p (x f) -> p x f", f=F2),
                    in_=PL2C[r0:r0 + BPC * 128, :]
                        .rearrange("(x p) f -> p x f", p=128))
                nc.scalar.dma_start(
                    out=PL2X[r0:r0 + BPC * 128, 0:F2]
                        .rearrange("(x p) f -> p x f", p=128),
                    in_=pk[:].rearrange("p (x f) -> p x f", f=F2))

            # ---- stage 4: layer-2 edge pass ----
            for b in range(BPC):
                g2 = gp.tile([128, CS2 * ROW], dt.bfloat16, tag="g2")
                g2v = g2[:].rearrange("p (c f) -> p c f", f=ROW)
                nc.gpsimd.dma_gather(
                    g2v[:, 0:CA2, :], PL2X[0:HALF, :],
                    ixa2[:, b * CA2 * 8:(b + 1) * CA2 * 8],
                    CA2 * 128, CA2 * 128, ROW, single_packet=False,
                    queue_num=(2 * b) % 4)
                nc.gpsimd.dma_gather(
                    g2v[:, CA2:CS2, :], PL2X[HALF:NP, :],
                    ixb2[:, b * CB2 * 8:(b + 1) * CB2 * 8],
                    CB2 * 128, CB2 * 128, ROW, single_packet=False,
                    queue_num=(2 * b + 1) % 4)

                P = bp.tile([128, CS2 * 128], dt.bfloat16, tag="P")
                nc.vector.tensor_tensor(
                    out=P[:].rearrange("p (c j) -> p c j", j=128),
                    in0=drel2[:, b * CS2:(b + 1) * CS2][:, :, None]
                        .broadcast_to([128, CS2, 128]),
                    in1=iota[:, None, :].broadcast_to([128, CS2, 128]),
                    op=AL.is_equal)
                drt = sp.tile([128, CS2 * 128], dt.uint8, tag="drt")
                nc.sync.dma_start(
                    out=drt[:],
                    in_=DRT2[b:b + 1, :].broadcast_to([128, CS2 * 128]))
                PT = bp.tile([128, CS2 * 128], dt.bfloat16, tag="PT")
                nc.vector.tensor_tensor(
                    out=PT[:].rearrange("p (c e) -> p c e", e=128),
                    in0=drt[:].rearrange("p (c e) -> p c e", e=128),
                    in1=iotap[:, :, None].broadcast_to([128, CS2, 128]),
                    op=AL.is_equal)

                ADG2 = pp.tile([128, CS2], dt.float32, tag="adg")
                for c in range(CS2):
                    nc.tensor.matmul(ADG2[:, c:c + 1],
                                     lhsT=PT[:, c * 128:(c + 1) * 128],
                                     rhs=adst2[:, b:b + 1],
                                     start=True, stop=True)

                elog2 = sp.tile([128, CS2], dt.float32, tag="elog2")
                nc.vector.tensor_tensor(
                    out=elog2[:, :, None], in0=ADG2[:, :, None],
                    in1=g2v[:, :, OUT_CH:OUT_CH + 1], op=AL.add)
                lr2 = sp.tile([128, CS2], dt.float32, tag="lr2")
                nc.vector.scalar_tensor_tensor(out=lr2[:], in0=elog2[:], scalar=NEG,
                                               in1=elog2[:], op0=AL.mult, op1=AL.max)
                s2 = sp.tile([128, CS2], dt.bfloat16, tag="s2")
                nc.scalar.activation(out=s2[:], in_=lr2[:], func=AF.Exp)

                gs2 = bp.tile([128, CS2 * OUT_CH], dt.bfloat16, tag="gs2")
                nc.vector.tensor_tensor(
                    out=gs2[:].rearrange("p (c f) -> p c f", f=OUT_CH),
                    in0=g2v[:, :, 0:OUT_CH],
                    in1=s2[:, :, None].broadcast_to([128, CS2, OUT_CH]),
                    op=AL.mult)

                AGG2T = pp.tile([128, OUT_CH], dt.float32, tag="agg")
                DEN2T = pp.tile([128, 1], dt.float32, tag="adg")
                for c in range(CS2):
                    nc.tensor.matmul(AGG2T[:],
                                     lhsT=P[:, c * 128:(c + 1) * 128],
                                     rhs=gs2[:, c * OUT_CH:(c + 1) * OUT_CH],
                                     start=(c == 0), stop=(c == CS2 - 1))
                    nc.tensor.matmul(DEN2T[:],
                                     lhsT=P[:, c * 128:(c + 1) * 128],
                                     rhs=s2[:, c:c + 1],
                                     start=(c == 0), stop=(c == CS2 - 1))

                rec2 = sp.tile([128, 1], dt.float32, tag="rec2")
                nc.vector.tensor_scalar(out=rec2[:], in0=DEN2T[:],
                                        scalar1=1e-16, scalar2=None, op0=AL.add)
                nc.vector.reciprocal_approx_fast(out=rec2[:], in_=rec2[:])
                o2 = sp.tile([128, OUT_CH], dt.float32, tag="o2")
                nc.vector.tensor_scalar(out=o2[:], in0=AGG2T[:],
                                        scalar1=rec2[:, 0:1], scalar2=None,
                                        op0=AL.mult)
                nc.sync.dma_start(out=OUT[b * 128:(b + 1) * 128, :], in_=o2[:])

    nc.compile()
    return nc


def _wrap_idx(lst):
    """list [n] (n % 128 == 0) -> [128, n//16] int16, wrapped + replicated."""
    S = len(lst) // 16
    w = np.asarray(lst, np.int16).reshape(S, 16).T       # [16, S]
    return np.tile(w, (8, 1))                            # [128, S]


def _host_prep(x, edge_index, W1, att_src1, att_dst1, W2, att_src2, att_dst2,
               n_nodes, n_edges):
    NBLK = -(-n_nodes // BLK)
    NBLK = -(-NBLK // NCORES) * NCORES
    NP = NBLK * BLK
    BPC = NBLK // NCORES

    x = np.asarray(x, np.float32)
    W1 = np.asarray(W1, np.float32)
    W2 = np.asarray(W2, np.float32)
    att_src1 = np.asarray(att_src1, np.float32)
    att_dst1 = np.asarray(att_dst1, np.float32)
    att_src2 = np.asarray(att_src2, np.float32)
    att_dst2 = np.asarray(att_dst2, np.float32)
    H, C = att_src1.shape

    xp = np.zeros((NP, IN_CH), np.float32)
    xp[:n_nodes] = x
    XT = np.ascontiguousarray(xp.T).astype(bf16)          # [128, NP]

    Asrc1 = np.zeros((H * C, H), np.float32)
    Adst1 = np.zeros((H * C, H), np.float32)
    for h in range(H):
        Asrc1[h * C:(h + 1) * C, h] = att_src1[h]
        Adst1[h * C:(h + 1) * C, h] = att_dst1[h]
    W1A = np.concatenate([W1, W1 @ Asrc1, W1 @ Adst1], axis=1).astype(bf16)
    W2A = np.concatenate([W2, W2 @ att_src2.T, W2 @ att_dst2.T], axis=1).astype(bf16)

    IOTA = np.ascontiguousarray(
        np.broadcast_to(np.arange(128, dtype=np.uint8), (128, 128)))
    IOTAP = np.arange(128, dtype=np.uint8)[:, None]
    IDB = np.eye(128, dtype=np.float32).astype(bf16)

    src = np.asarray(edge_index[0], np.int64)
    dst = np.asarray(edge_index[1], np.int64)
    order = np.lexsort((src, dst))
    ss = src[order]
    dd = dst[order]
    blk = dd // BLK
    core_of = blk // BPC
    b_of = blk % BPC
    d_rel = dd % BLK

    # per (core, block): L1 split on rotated ids, L2 split on global ids
    def rot_of(k, s):
        return ((s // BLK - k * BPC) % NBLK) * BLK + (s % BLK)

    # first pass: chunk maxima
    kA1 = np.zeros((NCORES, BPC), np.int64)
    kB1 = np.zeros((NCORES, BPC), np.int64)
    kA2 = np.zeros((NCORES, BPC), np.int64)
    kB2 = np.zeros((NCORES, BPC), np.int64)
    sel_cache = {}
    for k in range(NCORES):
        for b in range(BPC):
            m = (core_of == k) & (b_of == b)
            sel_cache[(k, b)] = m
            r = rot_of(k, ss[m])
            nA = int((r < HALF).sum()); nB = int(m.sum()) - nA
            kA1[k, b] = -(-nA // 128) if nA else 0
            kB1[k, b] = -(-nB // 128) if nB else 0
            nA2 = int((ss[m] < HALF).sum()); nB2 = int(m.sum()) - nA2
            kA2[k, b] = -(-nA2 // 128) if nA2 else 0
            kB2[k, b] = -(-nB2 // 128) if nB2 else 0
    CA1, CB1 = int(kA1.max()), int(kB1.max())
    CA2, CB2 = int(kA2.max()), int(kB2.max())
    CS1, CS2 = CA1 + CB1, CA2 + CB2

    IXA1 = np.zeros((NCORES, 128, BPC * CA1 * 8), np.int16)
    IXB1 = np.zeros((NCORES, 128, BPC * CB1 * 8), np.int16)
    IXA2 = np.zeros((NCORES, 128, BPC * CA2 * 8), np.int16)
    IXB2 = np.zeros((NCORES, 128, BPC * CB2 * 8), np.int16)
    DREL1 = np.full((NCORES, 128, BPC * CS1), 200, np.uint8)
    DREL2 = np.full((NCORES, 128, BPC * CS2), 200, np.uint8)
    DRT1 = np.full((NCORES, BPC, CS1 * 128), 200, np.uint8)
    DRT2 = np.full((NCORES, BPC, CS2 * 128), 200, np.uint8)
    XTs = []

    for k in range(NCORES):
        XTb = XT.reshape(128, NBLK, BLK)
        XTs.append(np.ascontiguousarray(
            np.roll(XTb, -k * BPC, axis=1).reshape(128, NP)))
        for b in range(BPC):
            m = sel_cache[(k, b)]
            s_k = ss[m]; dr = d_rel[m]
            rot = rot_of(k, s_k)
            # layer 1 (rotated split)
            a_m = rot < HALF
            for (mask, tbl_off, ix_arr, ca, c0) in (
                    (a_m, 0, IXA1, CA1, 0), (~a_m, HALF, IXB1, CB1, CA1)):
                idxs = rot[mask] - tbl_off
                drs = dr[mask]
                n = len(idxs)
                lst = np.zeros(ca * 128, np.int64)
                lst[:n] = idxs
                if ca:
                    ix_arr[k][:, b * ca * 8:(b + 1) * ca * 8] = _wrap_idx(lst)
                sl = np.arange(n)
                pp_, cc = sl % 128, sl // 128 + c0
                DREL1[k][pp_, b * CS1 + cc] = drs
                DRT1[k][b, cc * 128 + pp_] = drs
            # layer 2 (global split)
            a_m2 = s_k < HALF
            for (mask, tbl_off, ix_arr, ca, c0) in (
                    (a_m2, 0, IXA2, CA2, 0), (~a_m2, HALF, IXB2, CB2, CA2)):
                idxs = s_k[mask] - tbl_off
                drs = dr[mask]
                n = len(idxs)
                lst = np.zeros(ca * 128, np.int64)
                lst[:n] = idxs
                if ca:
                    ix_arr[k][:, b * ca * 8:(b + 1) * ca * 8] = _wrap_idx(lst)
                sl = np.arange(n)
                pp_, cc = sl % 128, sl // 128 + c0
                DREL2[k][pp_, b * CS2 + cc] = drs
                DRT2[k][b, cc * 128 + pp_] = drs

    consts = dict(w1a=W1A, w2a=W2A, iota=IOTA, iotap=IOTAP, idb=IDB)
    in_maps = []
    for k in range(NCORES):
        m = dict(consts)
        m["xt"] = XTs[k]
        m["ixa1"] = IXA1[k]; m["ixb1"] = IXB1[k]
        m["ixa2"] = IXA2[k]; m["ixb2"] = IXB2[k]
        m["drel1"] = DREL1[k]
        m["drel2"] = DREL2[k]
        m["drt1"] = DRT1[k]
        m["drt2"] = DRT2[k]
        in_maps.append(m)
    return NP, NBLK, BPC, (CA1, CB1, CA2, CB2), in_maps


_CACHE = {}


def _run(x, edge_index, W1, att_src1, att_dst1, W2, att_src2, att_dst2,
         n_nodes, n_edges, trace=False):
    from concourse import bass_utils
    NP, NBLK, BPC, CS, in_maps = _host_prep(
        x, edge_index, W1, att_src1, att_dst1, W2, att_src2, att_dst2,
        n_nodes, n_edges)
    key = (NP, CS)
    if key not in _CACHE:
        _CACHE[key] = _build(NP, NBLK, BPC, *CS)
    nc = _CACHE[key]
    res = bass_utils.run_bass_kernel_spmd(nc, in_maps, core_ids=list(range(NCORES)),
                                          trace=trace)
    out = np.concatenate([np.asarray(res.results[k]["out"]) for k in range(NCORES)],
                         axis=0)[:n_nodes]
    return np.ascontiguousarray(out.astype(np.float32)), res


def kernel(x, edge_index, W1, att_src1, att_dst1, W2, att_src2, att_dst2):
    out, _ = _run(x, edge_index, W1, att_src1, att_dst1, W2, att_src2, att_dst2,
                  N_NODES, N_EDGES)
    return out
